# revision 2
# baseline (speedup 1.0000x reference)
"""Trainium2 Bass kernel for nn_GroupQueryAttention_51616916963669.

GQA with YaRN RoPE, sliding-window (128) + causal mask, learned sink logit,
qkv/out projections. B=1, S=2048, E=2048, H=32, G=8, D=64.

Sharding over 8 NeuronCores: 2-way sequence (1024 queries each, with a
128-token KV halo) x 4-way heads (8 q-heads / 2 kv-groups each). Each core
computes a partial out-projection (over its 512 ctx dims); the host sums the
4 head-partials per sequence half and concatenates.

Device layout is feature-major throughout: x, weights are passed transposed
from the host so every matmul operand has its contraction dim on partitions.
Matmuls with free dim >= 256 use float32r (1 cycle/row vs 4 for fp32); the
probs @ V matmul runs in bf16. Softmax skips max-subtraction (scores are
O(10), fp32 exp is safe) and uses ScalarE activation accum_out for row sums.
DMAs are coalesced into single multi-dim-AP instructions to keep the SP
sequencer off the critical path.
"""
import numpy as np

# ---- problem constants (hardcoded per contract) ----
B, S, E = 1, 2048, 2048
H, G, D = 32, 8, 64
SW = 128
ROPE_BASE = 10000.0
ORIG_CTX = 4096.0
YARN_SCALE = 2.0
BETA_FAST, BETA_SLOW = 32.0, 1.0
NEG = -1e30

# ---- sharding constants ----
NCORES = 8
TOK = 1152           # local kv tokens (9 blocks of 128)
NQ = 1024            # local query tokens (kv blocks 1..8)
QH = 8               # q heads per core
KG = 2               # kv groups per core
FTOT = QH * D + 2 * KG * D   # 768
NE = E // 128        # 16 e-chunks
TCH = 384            # qkv matmul N-chunk
SCALE = 1.0 / (D ** 0.5)
PV_BF16 = True

_compiled = None


def _build_bass():
    import os
    PHASES = int(os.environ.get("KPHASES", "4"))
    import concourse.bacc as bacc
    import concourse.tile as tile
    import concourse.mybir as mybir
    from concourse.masks import make_identity

    f32 = mybir.dt.float32
    f32r = mybir.dt.float32r
    bf16 = mybir.dt.bfloat16
    pvdt = bf16 if PV_BF16 else f32
    Exp = mybir.ActivationFunctionType.Exp
    Copy = mybir.ActivationFunctionType.Identity

    nc = bacc.Bacc("TRN2", target_bir_lowering=False, debug=False,
                   num_devices=NCORES)

    xT = nc.dram_tensor("xT", [E, TOK], bf16, kind="ExternalInput").ap()
    wqkvT = nc.dram_tensor("wqkvT", [E, FTOT], bf16, kind="ExternalInput").ap()
    bqkvT = nc.dram_tensor("bqkvT", [128, FTOT // 128], f32, kind="ExternalInput").ap()
    woutT = nc.dram_tensor("woutT", [QH * D, E], bf16, kind="ExternalInput").ap()
    cosQ = nc.dram_tensor("cosQ", [128, TOK], f32, kind="ExternalInput").ap()
    sinQ = nc.dram_tensor("sinQ", [128, TOK], f32, kind="ExternalInput").ap()
    cosK = nc.dram_tensor("cosK", [128, TOK], f32, kind="ExternalInput").ap()
    sinK = nc.dram_tensor("sinK", [128, TOK], f32, kind="ExternalInput").ap()
    masksD = nc.dram_tensor("masks", [128, 2, 256], f32, kind="ExternalInput").ap()
    sinkb = nc.dram_tensor("sinkb", [128, QH], f32, kind="ExternalInput").ap()
    outT = nc.dram_tensor("outT", [E, NQ], f32, kind="ExternalOutput").ap()

    xT_r = xT.rearrange("(a p) t -> p a t", p=128)        # [128, 16, TOK]
    wq_r = wqkvT.rearrange("(a p) f -> p a f", p=128)     # [128, 16, 768]
    wo_r = woutT.rearrange("(a p) e -> p a e", p=128)     # [128, 4, 2048]
    out_r = outT.rearrange("(a p) q -> p a q", p=128)     # [128, 16, 1024]

    class _Done(Exception):
        pass

    with tile.TileContext(nc) as tc:
        from contextlib import ExitStack
        es = ExitStack()
        try:
          with es:
            persist = es.enter_context(tc.tile_pool(name="persist", bufs=1))
            qk_pool = es.enter_context(tc.tile_pool(name="qk", bufs=1))
            ctx_pool = es.enter_context(tc.tile_pool(name="ctx", bufs=1))
            qkv_pool = es.enter_context(tc.tile_pool(name="qkv", bufs=1))

            # persistent small tensors
            ident = persist.tile([128, 128], f32)
            make_identity(nc, ident)
            identb = persist.tile([128, 128], pvdt)
            nc.vector.tensor_copy(identb, ident)
            b_sb = persist.tile([128, FTOT // 128], f32)
            nc.sync.dma_start(b_sb, bqkvT)
            masks2 = persist.tile([128, 2, 256], f32)
            nc.sync.dma_start(masks2, masksD)
            sink_sb = persist.tile([128, QH], f32)
            nc.sync.dma_start(sink_sb, sinkb)
            es_sink = persist.tile([128, QH], f32)
            nc.scalar.activation(out=es_sink, in_=sink_sb, func=Exp)

            # qkvT result tiles (fp32): order K, V, Q0..Q3 for earliest K/V
            qkvT_t = [qkv_pool.tile([128, TOK], f32, tag=f"qkvT{i}", name=f"qkvT{i}")
                      for i in range(6)]
            F_ORDER = [4, 5, 0, 1, 2, 3]   # K, V first

            # ---- phase A: qkv projection ----
            with tc.tile_pool(name="phA", bufs=2) as pa, \
                 tc.tile_pool(name="phAw", bufs=1) as paw, \
                 tc.tile_pool(name="psA", bufs=3, space="PSUM") as psA:
                W_sb = paw.tile([128, NE, FTOT], bf16)
                for e in range(NE):
                    nc.sync.dma_start(W_sb[:, e, :], wq_r[:, e, :])
                for t in range(TOK // TCH):
                    x_sb = pa.tile([128, NE, TCH], bf16, tag="x")
                    for e in range(NE):
                        nc.sync.dma_start(
                            x_sb[:, e, :], xT_r[:, e, TCH * t:TCH * (t + 1)])
                    for f in F_ORDER:
                        ps = psA.tile([128, TCH], f32, tag="mmA")
                        for e in range(NE):
                            nc.tensor.matmul(
                                ps, W_sb[:, e, 128 * f:128 * (f + 1)], x_sb[:, e, :],
                                start=(e == 0), stop=(e == NE - 1))
                        nc.scalar.activation(
                            out=qkvT_t[f][:, TCH * t:TCH * (t + 1)], in_=ps,
                            func=Copy, bias=b_sb[:, f:f + 1])

            # ---- phase A2: RoPE (K first), then V transpose, then Q ----
            if PHASES < 2:
                raise _Done()
            QR = [qk_pool.tile([128, TOK], f32r, tag=f"QR{i}", name=f"QR{i}")
                  for i in range(4)]
            KR = qk_pool.tile([128, TOK], f32r, tag="KR")
            KRsw = qk_pool.tile([128, TOK], f32r, tag="KRsw")
            Vtok = ctx_pool.tile([128, 9, KG, D], pvdt)
            with tc.tile_pool(name="rope", bufs=2) as pr, \
                 tc.tile_pool(name="cs", bufs=1) as pcs, \
                 tc.tile_pool(name="psT", bufs=2, space="PSUM") as psT, \
                 tc.tile_pool(name="vtd", bufs=2) as _vtd:
                cQ = pcs.tile([128, TOK], f32)
                sQ = pcs.tile([128, TOK], f32)
                cK = pcs.tile([128, TOK], f32)
                sK = pcs.tile([128, TOK], f32)
                nc.sync.dma_start(cQ, cosQ)
                nc.sync.dma_start(sQ, sinQ)
                nc.sync.dma_start(cK, cosK)
                nc.sync.dma_start(sK, sinK)

                def rope(src, cT, sT, dst, t):
                    # one 384-column chunk: unblocks consumers incrementally
                    cs_ = slice(TCH * t, TCH * (t + 1))
                    r = pr.tile([128, TCH], f32, tag="rot", name="rot")
                    nc.gpsimd.tensor_copy(r[0:32, :], src[32:64, cs_])
                    nc.scalar.copy(r[32:64, :], src[0:32, cs_])
                    nc.gpsimd.tensor_copy(r[64:96, :], src[96:128, cs_])
                    nc.scalar.copy(r[96:128, :], src[64:96, cs_])
                    a = pr.tile([128, TCH], f32, tag="a", name="a")
                    nc.vector.tensor_mul(a, src[:, cs_], cT[:, cs_])
                    nc.vector.tensor_mul(r, r, sT[:, cs_])
                    nc.gpsimd.tensor_add(dst[:, cs_], a, r)

                V = qkvT_t[5]
                for t in range(TOK // TCH):
                    rope(qkvT_t[4], cK, sK, KR, t)
                    nc.gpsimd.tensor_copy(
                        KRsw[0:64, TCH * t:TCH * (t + 1)],
                        KR[64:128, TCH * t:TCH * (t + 1)])
                    nc.gpsimd.tensor_copy(
                        KRsw[64:128, TCH * t:TCH * (t + 1)],
                        KR[0:64, TCH * t:TCH * (t + 1)])
                    for k in range(3 * t, 3 * (t + 1)):
                        for g in range(KG):
                            pt = psT.tile([128, D], f32, tag="vt", name="vt")
                            nc.tensor.transpose(
                                pt, V[64 * g:64 * (g + 1), 128 * k:128 * (k + 1)],
                                ident[64 * g:64 * (g + 1), 64 * g:64 * (g + 1)])
                            nc.vector.tensor_copy(Vtok[:, k, g, :], pt)
                for i in range(4):
                    for t in range(TOK // TCH):
                        rope(qkvT_t[i], cQ, sQ, QR[i], t)

            if PHASES < 3:
                raise _Done()
            # ---- phase B: attention ----
            # ctxT split by query half so out-proj can start early
            ctxT = [[ctx_pool.tile([128, 512], bf16, tag=f"ctxT{i}_{th}",
                                   name=f"ctxT{i}_{th}") for th in range(2)]
                    for i in range(4)]
            with tc.tile_pool(name="phB", bufs=4) as pb, \
                 tc.tile_pool(name="psSC", bufs=3, space="PSUM") as psSC, \
                 tc.tile_pool(name="psTP", bufs=2, space="PSUM") as psTP, \
                 tc.tile_pool(name="psPV", bufs=3, space="PSUM") as psPV:
                for qb in range(8):
                    for hh in range(QH):
                        g = hh // 4
                        half = hh % 2
                        qtile = QR[hh // 2]
                        if half == 0:
                            ktile = KR if g == 0 else KRsw
                        else:
                            ktile = KRsw if g == 0 else KR
                        qsl_all = qtile[64 * half:64 * (half + 1), :]
                        ksl_all = ktile[64 * half:64 * (half + 1), :]
                        sc = psSC.tile([128, 256], f32, tag="sc", name="sc")
                        nc.tensor.matmul(
                            sc,
                            qsl_all[:, 128 * (qb + 1):128 * (qb + 2)],
                            ksl_all[:, 128 * qb:128 * qb + 256],
                            start=True, stop=True)
                        sm = pb.tile([128, 256], f32, tag="sm", name="sm")
                        nc.vector.tensor_add(sm, sc, masks2[:, min(qb, 1), :])
                        pS = pb.tile([128, 256], f32, tag="pS", name="pS")
                        rs = pb.tile([128, 1], f32, tag="rs", name="rs")
                        nc.scalar.activation(out=pS, in_=sm, func=Exp, accum_out=rs)
                        dn = pb.tile([128, 1], f32, tag="dn", name="dn")
                        nc.vector.tensor_scalar_add(dn, rs, es_sink[:, hh:hh + 1])
                        rinv = pb.tile([128, 1], f32, tag="rinv", name="rinv")
                        nc.vector.reciprocal(rinv, dn)
                        pN = pb.tile([128, 256], pvdt, tag="pN", name="pN")
                        nc.scalar.activation(out=pN, in_=pS, func=Copy, scale=rinv)
                        pT = pb.tile([128, 256], pvdt, tag="pT", name="pT")
                        for hb in range(2):
                            tp = psTP.tile([128, 128], pvdt, tag="tp", name="tp")
                            nc.tensor.transpose(
                                tp, pN[:, 128 * hb:128 * (hb + 1)], identb)
                            nc.vector.tensor_copy(pT[:, 128 * hb:128 * (hb + 1)], tp)
                        cps = psPV.tile([64, 128], f32, tag="pv", name="pv")
                        nc.tensor.matmul(cps, Vtok[:, qb, g, :], pT[:, 0:128],
                                         start=True, stop=False)
                        nc.tensor.matmul(cps, Vtok[:, qb + 1, g, :], pT[:, 128:256],
                                         start=False, stop=True)
                        nc.vector.tensor_copy(
                            ctxT[hh // 2][qb // 4][64 * half:64 * (half + 1),
                                                   128 * (qb % 4):128 * (qb % 4 + 1)],
                            cps)

            # ---- phase C: out projection (partial over this core's 512 dims) ----
            with tc.tile_pool(name="phC", bufs=1) as pc, \
                 tc.tile_pool(name="phCo", bufs=3) as pco, \
                 tc.tile_pool(name="psC", bufs=3, space="PSUM") as psC:
                Wo = pc.tile([128, 4, E], bf16)
                for h4 in range(4):
                    nc.sync.dma_start(Wo[:, h4, :], wo_r[:, h4, :])
                for t in range(2):
                    for e in range(NE):
                        ps = psC.tile([128, 512], f32, tag="mmC", name="mmC")
                        for h4 in range(4):
                            nc.tensor.matmul(
                                ps, Wo[:, h4, 128 * e:128 * (e + 1)],
                                ctxT[h4][t],
                                start=(h4 == 0), stop=(h4 == 3))
                        o_sb = pco.tile([128, 512], f32, tag="o", name="o")
                        nc.vector.tensor_copy(o_sb, ps)
                        nc.sync.dma_start(
                            out_r[:, e, 512 * t:512 * (t + 1)], o_sb)

        except _Done:
            pass
    nc.compile()
    return nc


# ---------------- host-side prep ----------------

def _rope_tables(position_ids, gstart):
    pos = np.zeros(TOK, dtype=np.float32)
    idx = gstart + np.arange(TOK)
    valid = (idx >= 0) & (idx < S)
    pos[valid] = position_ids[0, idx[valid]].astype(np.float32)
    freqs = (1.0 / ROPE_BASE ** (np.arange(0, D, 2, dtype=np.float32) / D)).astype(np.float32)
    wave_len = 2.0 * np.pi / freqs
    low = ORIG_CTX / BETA_FAST
    high = ORIG_CTX / BETA_SLOW
    t = np.clip((wave_len - low) / (high - low), 0.0, 1.0)
    eff = freqs * (1.0 - t) + (freqs / YARN_SCALE) * t
    conc = 0.1 * np.log(np.float32(YARN_SCALE)) + 1.0
    ang = pos[:, None] * eff[None, :] * conc
    sin = np.sin(ang).astype(np.float32).T    # [32, TOK]
    cos = np.cos(ang).astype(np.float32).T
    cosT = np.concatenate([cos, cos], axis=0)  # [64, TOK]
    sinS = np.concatenate([-sin, sin], axis=0)
    cos2 = np.concatenate([cosT, cosT], axis=0)  # [128, TOK]
    sinS2 = np.concatenate([sinS, sinS], axis=0)
    return np.ascontiguousarray(cos2), np.ascontiguousarray(sinS2)


def _build_masks(attn_mask, s, gstart):
    qb = np.arange(2)[None, :, None]
    il = np.arange(128)[:, None, None]
    j = np.arange(256)[None, None, :]
    gq = 1024 * s + 128 * qb + il
    gk = gstart + 128 * qb + j
    gq_b, gk_b = np.broadcast_arrays(gq, gk)
    valid = (gk_b >= 0) & (gk_b <= gq_b) & (gk_b > gq_b - SW)
    base = np.where(
        valid,
        np.maximum(attn_mask[0, 0, gq_b, np.clip(gk_b, 0, S - 1)], NEG),
        NEG)
    return np.ascontiguousarray(base.astype(np.float32))


def _prep_core(c, x, position_ids, attn_mask, Wqkv, bqkv, Wout, sinks, xT_full):
    s, h = c // 4, c % 4
    gstart = 1024 * s - 128
    xTc = np.zeros((E, TOK), dtype=np.float32)
    lo = max(0, gstart)
    xTc[:, lo - gstart:TOK] = xT_full[:, lo:gstart + TOK]
    qrows = np.arange(512 * h, 512 * h + 512)
    krows = np.arange(H * D + 128 * h, H * D + 128 * h + 128)
    vrows = np.arange((H + G) * D + 128 * h, (H + G) * D + 128 * h + 128)
    rows = np.concatenate([qrows, krows, vrows])
    WqkvTc = np.ascontiguousarray(Wqkv[rows].T)
    bq = bqkv[rows].reshape(FTOT // 128, 128).T
    WoutTc = np.ascontiguousarray(Wout[:, 512 * h:512 * h + 512].T)
    cos2, sinS2 = _rope_tables(position_ids, gstart)
    masks = _build_masks(attn_mask, s, gstart)
    sink_c = np.ascontiguousarray(
        np.broadcast_to(sinks[0, 8 * h:8 * h + 8, 0, 0][None, :], (128, QH))
    ).astype(np.float32)
    import ml_dtypes
    bf = ml_dtypes.bfloat16
    return {
        "xT": np.ascontiguousarray(xTc.astype(bf)),
        "wqkvT": np.ascontiguousarray(WqkvTc.astype(bf)),
        "bqkvT": np.ascontiguousarray(bq.astype(np.float32)),
        "woutT": np.ascontiguousarray(WoutTc.astype(bf)),
        "cosQ": np.ascontiguousarray(SCALE * cos2),
        "sinQ": np.ascontiguousarray(SCALE * sinS2),
        "cosK": cos2,
        "sinK": sinS2,
        "masks": masks,
        "sinkb": sink_c,
    }


def _prep_all(inputs):
    x = np.asarray(inputs["x"], dtype=np.float32)
    position_ids = np.asarray(inputs["position_ids"])
    attn_mask = np.asarray(inputs["attn_mask"], dtype=np.float32)
    Wqkv = np.asarray(inputs["Wqkv"], dtype=np.float32)
    bqkv = np.asarray(inputs["bqkv"], dtype=np.float32)
    Wout = np.asarray(inputs["Wout"], dtype=np.float32)
    sinks = np.asarray(inputs["sinks"], dtype=np.float32)
    xT_full = np.ascontiguousarray(x[0].T)
    return [
        _prep_core(c, x, position_ids, attn_mask, Wqkv, bqkv, Wout, sinks, xT_full)
        for c in range(NCORES)
    ]


def kernel(x, position_ids, attn_mask, Wqkv, bqkv, Wout, bout, sinks):
    global _compiled
    from concourse.bass_utils import run_bass_kernel_spmd

    bout = np.asarray(bout, dtype=np.float32)

    if _compiled is None:
        _compiled = _build_bass()
    nc = _compiled

    in_maps = _prep_all({
        "x": x, "position_ids": position_ids, "attn_mask": attn_mask,
        "Wqkv": Wqkv, "bqkv": bqkv, "Wout": Wout, "bout": bout, "sinks": sinks,
    })
    res = run_bass_kernel_spmd(nc, in_maps, list(range(NCORES)))

    out = np.empty((S, E), dtype=np.float32)
    for s in range(2):
        acc = res.results[4 * s]["outT"].astype(np.float32).copy()
        for h in range(1, 4):
            acc += res.results[4 * s + h]["outT"]
        out[1024 * s:1024 * (s + 1)] = acc.T
    out += bout[None, :]
    return out[None]



# revision 37
# speedup vs baseline: 1.4182x; 1.4182x over previous
"""Trainium2 Bass kernel for nn_GroupQueryAttention_51616916963669.

GQA with YaRN RoPE, sliding-window (128) + causal mask, learned sink logit,
qkv/out projections. B=1, S=2048, E=2048, H=32, G=8, D=64.

Sharding over 8 NeuronCores: 2-way sequence (1024 queries each, with a
128-token KV halo) x 4-way heads (8 q-heads / 2 kv-groups each). Each core
computes a partial out-projection (over its 512 ctx dims); the host sums the
4 head-partials per sequence half and concatenates.

v2 design notes:
- qkv matmul streams 3x384 columns per stationary load (LDWEIGHTS amortized),
  with W/x DMAs interleaved per contraction chunk so compute starts ~2us in.
- RoPE rotate-half is a PE permutation matmul (f32r, 1 cyc/row); the
  cos/sin multiply-adds run on DVE. Nothing touches the slow GpSimd engine.
- Softmax: exp runs unmasked straight out of PSUM (one big ACT op per 4-head
  group); the 0/1 band mask + row sums come from one fused DVE
  scalar_tensor_tensor with accum_out; normalization is a per-head
  tensor_scalar into bf16. Probs are transposed by the XBAR DMA engine
  (dma_start_transpose) rather than PE transposes + PSUM round trips.
- Out-projection is emitted per query-half so its matmuls overlap the second
  half of attention; outputs DMA back in bf16.
"""
import os
import numpy as np

# ---- problem constants (hardcoded per contract) ----
B, S, E = 1, 2048, 2048
H, G, D = 32, 8, 64
SW = 128
ROPE_BASE = 10000.0
ORIG_CTX = 4096.0
YARN_SCALE = 2.0
BETA_FAST, BETA_SLOW = 32.0, 1.0

# ---- sharding constants ----
NCORES = 8
TOK = 1152           # local kv tokens (9 blocks of 128)
NQ = 1024            # local query tokens (kv blocks 1..8)
QH = 8               # q heads per core
KG = 2               # kv groups per core
FTOT = QH * D + 2 * KG * D   # 768, feature order [K, V, Q0..Q3]
NE = E // 128        # 16 e-chunks
TCH = 384            # qkv matmul N-chunk
NT = TOK // TCH      # 3
SCALE = 1.0 / (D ** 0.5)

USE_DMA_TP = os.environ.get("KDMATP", "0") == "1"
USE_STT = os.environ.get("KSTT", "1") == "1"
KSTAGE = int(os.environ.get("KSTAGE", "3"))
KB = int(os.environ.get("KB", "9"))  # attn_group sub-stage bisect

_compiled = None


def _build_bass():
    import concourse.bacc as bacc
    import concourse.tile as tile
    import concourse.mybir as mybir
    from concourse.masks import make_identity

    f32 = mybir.dt.float32
    f32r = mybir.dt.float32r
    bf16 = mybir.dt.bfloat16
    Exp = mybir.ActivationFunctionType.Exp
    Ident = mybir.ActivationFunctionType.Identity
    Alu = mybir.AluOpType

    nc = bacc.Bacc("TRN2", target_bir_lowering=False, debug=False,
                   num_devices=NCORES)

    xT = nc.dram_tensor("xT", [E, TOK], bf16, kind="ExternalInput").ap()
    wqkvT = nc.dram_tensor("wqkvT", [E, FTOT], bf16, kind="ExternalInput").ap()
    bqkvT = nc.dram_tensor("bqkvT", [128, FTOT // 128], f32, kind="ExternalInput").ap()
    woutT = nc.dram_tensor("woutT", [QH * D, E], bf16, kind="ExternalInput").ap()
    cosQ = nc.dram_tensor("cosQ", [128, TOK], bf16, kind="ExternalInput").ap()
    sinQ = nc.dram_tensor("sinQ", [128, TOK], bf16, kind="ExternalInput").ap()
    cosK = nc.dram_tensor("cosK", [128, TOK], bf16, kind="ExternalInput").ap()
    sinK = nc.dram_tensor("sinK", [128, TOK], bf16, kind="ExternalInput").ap()
    masksD = nc.dram_tensor("masks", [128, 2, 256], f32, kind="ExternalInput").ap()
    esinkD = nc.dram_tensor("esink", [128, QH], f32, kind="ExternalInput").ap()
    permD = nc.dram_tensor("perm", [128, 128], f32r, kind="ExternalInput").ap()
    outT = nc.dram_tensor("outT", [E, NQ], bf16, kind="ExternalOutput").ap()

    xT_r = xT.rearrange("(a p) t -> p a t", p=128)        # [128, 16, TOK]
    wq_r = wqkvT.rearrange("(a p) f -> p a f", p=128)     # [128, 16, 768]
    wo_r = woutT.rearrange("(a p) e -> p a e", p=128)     # [128, 4, 2048]
    out_r = outT.rearrange("(a p) q -> p a q", p=128)     # [128, 16, 1024]

    class _Done(Exception):
        pass

    with tile.TileContext(nc) as tc:
        from contextlib import ExitStack
        es = ExitStack()
        try:
          with es:
            persist = es.enter_context(tc.tile_pool(name="persist", bufs=1))
            qk_pool = es.enter_context(tc.tile_pool(name="qk", bufs=1))
            ctx_pool = es.enter_context(tc.tile_pool(name="ctx", bufs=1))
            qkv_pool = es.enter_context(tc.tile_pool(name="qkv", bufs=1))
            inp_pool = es.enter_context(tc.tile_pool(name="inp", bufs=1))

            # ---- persistent small tensors + big input buffers ----
            W_sb = inp_pool.tile([128, NE, FTOT], bf16)
            x_sb = inp_pool.tile([128, NE, TOK], bf16)
            # DMA order: per-e W chunk then x chunk, feature-group-major so
            # the accumulation loop's operands land just in time.
            for e in range(NE):
                nc.sync.dma_start(W_sb[:, e, 0:256], wq_r[:, e, 0:256])
                nc.sync.dma_start(x_sb[:, e, :], xT_r[:, e, :])
            for fg in range(1, 3):
                for e in range(NE):
                    nc.sync.dma_start(W_sb[:, e, 256 * fg:256 * (fg + 1)],
                                      wq_r[:, e, 256 * fg:256 * (fg + 1)])

            ident = persist.tile([128, 128], f32)
            make_identity(nc, ident)
            identb = persist.tile([128, 128], bf16)
            nc.vector.tensor_copy(identb, ident)
            b_sb = persist.tile([128, FTOT // 128], f32)
            nc.sync.dma_start(b_sb, bqkvT)
            masks2 = persist.tile([128, 2, 256], f32)
            nc.sync.dma_start(masks2, masksD)
            es_sink = persist.tile([128, QH], f32)
            nc.sync.dma_start(es_sink, esinkD)
            perm = persist.tile([128, 128], f32r)
            nc.sync.dma_start(perm, permD)
            cs_t = {}
            for nm, src in (("cQ", cosQ), ("sQ", sinQ), ("cK", cosK), ("sK", sinK)):
                t = persist.tile([128, TOK], bf16, tag=nm, name=nm)
                nc.sync.dma_start(t, src)
                cs_t[nm] = t
            Wo = persist.tile([128, 4, E], bf16)
            for h4 in range(4):
                nc.sync.dma_start(Wo[:, h4, :], wo_r[:, h4, :])

            # qkv projection results (feature blocks: 0=K, 1=V, 2..5=Q0..Q3)
            # f32r so the rotate-half / scores matmuls can consume directly
            # (V stays f32: it only feeds the PE transpose).
            qkvT_t = [qkv_pool.tile([128, TOK], f32 if i == 1 else f32r,
                                    tag=f"qkvT{i}", name=f"qkvT{i}")
                      for i in range(6)]
            QR = [qk_pool.tile([128, TOK], f32r, tag=f"QR{i}", name=f"QR{i}")
                  for i in range(4)]
            KR = qk_pool.tile([128, TOK], f32r, tag="KR")
            KRsw = qk_pool.tile([128, TOK], f32r, tag="KRsw")
            Vtok = ctx_pool.tile([128, 9, KG, D], bf16)
            # ctx transposed: [128 part = pair of heads, pair-idx 4, q-half 2, 512]
            ctxT = ctx_pool.tile([128, 4, 2, 512], bf16)

            esA = ExitStack()
            psA = esA.enter_context(
                tc.tile_pool(name="psA", bufs=2, space="PSUM"))
            psR = esA.enter_context(
                tc.tile_pool(name="psR", bufs=2, space="PSUM"))
            rope_sc = es.enter_context(tc.tile_pool(name="ropesc", bufs=3))

            def qkv_block(f):
                """Accumulate feature block f over all 16 e-chunks; one
                stationary load per (e), streaming 3x384 columns."""
                pst = [psA.tile([128, TCH], f32, tag=f"mmA{t}", name=f"mmA{t}")
                       for t in range(NT)]
                for e in range(NE):
                    for t in range(NT):
                        nc.tensor.matmul(
                            pst[t], W_sb[:, e, 128 * f:128 * (f + 1)],
                            x_sb[:, e, TCH * t:TCH * (t + 1)],
                            start=(e == 0), stop=(e == NE - 1))
                for t in range(NT):
                    nc.scalar.activation(
                        out=qkvT_t[f][:, TCH * t:TCH * (t + 1)], in_=pst[t],
                        func=Ident, bias=b_sb[:, f:f + 1])

            def rope(f, cT, sT, dst, also_swap=None):
                """dst = qkvT[f]*cos + (perm @ qkvT[f])*sinS, in 384-col
                chunks. Rotate-half runs on PE (f32r perm matmul)."""
                src = qkvT_t[f]
                for t in range(NT):
                    cs_ = slice(TCH * t, TCH * (t + 1))
                    rot = psR.tile([128, TCH], f32, tag="rot", name="rot")
                    nc.tensor.matmul(rot, perm, src[:, cs_],
                                     start=True, stop=True)
                    m1 = rope_sc.tile([128, TCH], f32, tag="m1", name="m1")
                    nc.vector.tensor_mul(m1, src[:, cs_], cT[:, cs_])
                    m2 = rope_sc.tile([128, TCH], f32, tag="m2", name="m2")
                    nc.vector.tensor_mul(m2, rot, sT[:, cs_])
                    nc.vector.tensor_add(dst[:, cs_], m1, m2)
                    if also_swap is not None:
                        nc.vector.tensor_add(
                            also_swap[0:64, cs_], m1[64:128, :], m2[64:128, :])
                        nc.vector.tensor_add(
                            also_swap[64:128, cs_], m1[0:64, :], m2[0:64, :])

            def v_transpose():
                V = qkvT_t[1]
                for k in range(9):
                    for g in range(KG):
                        pt = psR.tile([128, TCH], f32, tag="rot", name="vt")
                        nc.tensor.transpose(
                            pt[:, 0:D],
                            V[64 * g:64 * (g + 1), 128 * k:128 * (k + 1)],
                            ident[64 * g:64 * (g + 1), 64 * g:64 * (g + 1)])
                        nc.vector.tensor_copy(Vtok[:, k, g, :], pt[:, 0:D])

            pb = es.enter_context(tc.tile_pool(name="phB", bufs=2))
            pbt = es.enter_context(tc.tile_pool(name="phBt", bufs=4))

            psB = psBc = psBt = None
            last_t = {}

            def attn_group(qb, g):
                """One 4-head group (kv group g) for query block qb."""
                if KB <= 0:
                    last_t["d"] = QR[3][:, 0:512]
                    return
                # Slot order [4g, 4g+2, 4g+1, 4g+3]: each PSUM bank gets a
                # same-half pair (consecutive f32r matmuls must not switch
                # operand partition offset within one PSUM bank — HW fault).
                sc4 = psB.tile([128, 4, 256], f32, tag="sc4", name="sc4")
                for slot in range(4):
                    half = slot // 2
                    pair = 2 * g + (slot % 2)
                    ktile = KR if (g == half) else KRsw
                    qsl = QR[pair][64 * half:64 * (half + 1), :]
                    ksl = ktile[64 * half:64 * (half + 1), :]
                    nc.tensor.matmul(
                        sc4[:, slot, :],
                        qsl[:, 128 * (qb + 1):128 * (qb + 2)],
                        ksl[:, 128 * qb:128 * qb + 256],
                        start=True, stop=True)
                pS4 = pb.tile([128, 4, 256], f32, tag="pS4", name="pS4")
                # two ops: each reads a single 2KB PSUM bank
                nc.scalar.activation(out=pS4[:, 0:2, :], in_=sc4[:, 0:2, :],
                                     func=Exp)
                nc.scalar.activation(out=pS4[:, 2:4, :], in_=sc4[:, 2:4, :],
                                     func=Exp)
                last_t["d"] = pS4[:, 0:2, :]
                if KB <= 1:
                    return
                pM4 = pb.tile([128, 4, 256], f32, tag="pM4", name="pM4")
                rs4 = pb.tile([128, 4], f32, tag="rs4", name="rs4")
                mk = masks2[:, min(qb, 1), :]
                if USE_STT:
                    for j in range(4):
                        nc.vector.scalar_tensor_tensor(
                            out=pM4[:, j, :], in0=pS4[:, j, :], scalar=0.0,
                            in1=mk, op0=Alu.bypass, op1=Alu.mult,
                            accum_out=rs4[:, j:j + 1])
                else:
                    for j in range(4):
                        nc.vector.tensor_mul(pM4[:, j, :], pS4[:, j, :], mk)
                    nc.vector.tensor_reduce(
                        rs4, pM4, axis=mybir.AxisListType.X, op=Alu.add)
                last_t["d"] = pM4[:, 0:2, :]
                if KB <= 2:
                    return
                dn4 = pb.tile([128, 4], f32, tag="dn4", name="dn4")
                nc.vector.tensor_add(dn4, rs4, es_sink[:, 4 * g:4 * g + 4])
                rinv4 = pb.tile([128, 4], f32, tag="rinv4", name="rinv4")
                nc.vector.reciprocal(rinv4, dn4)
                pN4 = pb.tile([128, 4, 256], bf16, tag="pN4", name="pN4")
                for j in range(4):
                    nc.vector.tensor_scalar_mul(
                        pN4[:, j, :], pM4[:, j, :], rinv4[:, j:j + 1])
                last_t["d"] = pN4[:, 0:2, :]
                if KB <= 3:
                    return
                pT4 = pbt.tile([128, 4, 2, 128], bf16, tag="pT4", name="pT4")
                if USE_DMA_TP:
                    for j in range(4):
                        for bk in range(2):
                            eng = nc.sync if (j + bk) % 2 == 0 else nc.scalar
                            eng.dma_start_transpose(
                                pT4[:, j, bk, :],
                                pN4[:, j, 128 * bk:128 * (bk + 1)])
                else:
                    tp4 = psBt.tile([128, 4, 2, 128], bf16, tag="tp4",
                                    name="tp4")
                    for j in range(4):
                        for bk in range(2):
                            nc.tensor.transpose(
                                tp4[:, j, bk, :],
                                pN4[:, j, 128 * bk:128 * (bk + 1)], identb)
                    nc.vector.tensor_copy(pT4, tp4)
                last_t["d"] = pT4[:, 0:2, 0, :]
                if KB <= 4:
                    return
                cps4 = psBc.tile([64, 4, 128], f32, tag="cps4", name="cps4")
                for j in range(4):
                    for bk in range(2):
                        nc.tensor.matmul(
                            cps4[:, j, :], Vtok[:, qb + bk, g, :],
                            pT4[:, j, bk, :],
                            start=(bk == 0), stop=(bk == 1))
                th, qq = qb // 4, qb % 4
                nc.vector.tensor_copy(
                    ctxT[0:64, 2 * g:2 * g + 2, th, 128 * qq:128 * (qq + 1)],
                    cps4[:, 0:2, :])
                nc.vector.tensor_copy(
                    ctxT[64:128, 2 * g:2 * g + 2, th, 128 * qq:128 * (qq + 1)],
                    cps4[:, 2:4, :])

            pco = es.enter_context(tc.tile_pool(name="phCo", bufs=3))
            psC = None

            def outproj_half(t):
                for e in range(NE):
                    ps = psC.tile([128, 512], f32, tag="mmC", name="mmC")
                    for h4 in range(4):
                        nc.tensor.matmul(
                            ps, Wo[:, h4, 128 * e:128 * (e + 1)],
                            ctxT[:, h4, t, :],
                            start=(h4 == 0), stop=(h4 == 3))
                    o_sb = pco.tile([128, 512], bf16, tag="o", name="o")
                    nc.vector.tensor_copy(o_sb, ps)
                    nc.sync.dma_start(
                        out_r[:, e, 512 * t:512 * (t + 1)], o_sb)

            # ---------- emission schedule ----------
            qkv_block(0)                                   # K
            rope(0, cs_t["cK"], cs_t["sK"], KR, also_swap=KRsw)
            qkv_block(1)                                   # V
            v_transpose()
            qkv_block(2)                                   # Q0
            rope(2, cs_t["cQ"], cs_t["sQ"], QR[0])
            qkv_block(3)                                   # Q1
            rope(3, cs_t["cQ"], cs_t["sQ"], QR[1])
            qkv_block(4)                                   # Q2
            rope(4, cs_t["cQ"], cs_t["sQ"], QR[2])
            qkv_block(5)                                   # Q3
            rope(5, cs_t["cQ"], cs_t["sQ"], QR[3])
            esA.close()                                    # free A PSUM banks

            if KSTAGE <= 1:
                # debug: dump QR3 low columns so the kernel has an output
                dbg = pco.tile([128, 512], bf16, tag="o", name="o")
                nc.vector.tensor_copy(dbg, QR[3][:, 0:512])
                nc.sync.dma_start(out_r[:, 0, 0:512], dbg)
                raise _Done()

            psB = es.enter_context(
                tc.tile_pool(name="psB", bufs=2 if USE_DMA_TP else 1,
                             space="PSUM"))
            psBc = es.enter_context(
                tc.tile_pool(name="psBc", bufs=2, space="PSUM"))
            if not USE_DMA_TP:
                psBt = es.enter_context(
                    tc.tile_pool(name="psBt", bufs=2, space="PSUM"))
            psC = es.enter_context(
                tc.tile_pool(name="psC", bufs=2, space="PSUM"))

            for qb in range(8):                            # attention, group 0
                attn_group(qb, 0)
            for qb in range(4):                            # attention, group 1
                attn_group(qb, 1)
            if KSTAGE <= 2:
                dbg = pco.tile([128, 512], bf16, tag="o", name="o")
                src = ctxT[:, 0, 0, :] if KB >= 5 else last_t["d"]
                nc.vector.tensor_copy(dbg, src)
                nc.sync.dma_start(out_r[:, 0, 0:512], dbg)
                raise _Done()
            outproj_half(0)
            for qb in range(4, 8):
                attn_group(qb, 1)
            outproj_half(1)
        except _Done:
            pass

    nc.compile()
    return nc


# ---------------- host-side prep ----------------

def _rope_tables(position_ids, gstart):
    pos = np.zeros(TOK, dtype=np.float32)
    idx = gstart + np.arange(TOK)
    valid = (idx >= 0) & (idx < S)
    pos[valid] = position_ids[0, idx[valid]].astype(np.float32)
    freqs = (1.0 / ROPE_BASE ** (np.arange(0, D, 2, dtype=np.float32) / D)).astype(np.float32)
    wave_len = 2.0 * np.pi / freqs
    low = ORIG_CTX / BETA_FAST
    high = ORIG_CTX / BETA_SLOW
    t = np.clip((wave_len - low) / (high - low), 0.0, 1.0)
    eff = freqs * (1.0 - t) + (freqs / YARN_SCALE) * t
    conc = 0.1 * np.log(np.float32(YARN_SCALE)) + 1.0
    ang = pos[:, None] * eff[None, :] * conc
    sin = np.sin(ang).astype(np.float32).T    # [32, TOK]
    cos = np.cos(ang).astype(np.float32).T
    cosT = np.concatenate([cos, cos], axis=0)  # [64, TOK]
    sinS = np.concatenate([-sin, sin], axis=0)
    cos2 = np.concatenate([cosT, cosT], axis=0)  # [128, TOK]
    sinS2 = np.concatenate([sinS, sinS], axis=0)
    return np.ascontiguousarray(cos2), np.ascontiguousarray(sinS2)


def _build_masks01(s, gstart):
    """Multiplicative 0/1 band mask, [128, 2, 256] (qb==0 variant, qb>=1)."""
    qb = np.arange(2)[None, :, None]
    il = np.arange(128)[:, None, None]
    j = np.arange(256)[None, None, :]
    gq = 1024 * s + 128 * qb + il
    gk = gstart + 128 * qb + j
    gq_b, gk_b = np.broadcast_arrays(gq, gk)
    valid = (gk_b >= 0) & (gk_b <= gq_b) & (gk_b > gq_b - SW)
    return np.ascontiguousarray(valid.astype(np.float32))


def _perm_matrix():
    """lhsT for rotate-half: out[p] = src[p xor 32] within each 64-half."""
    P = np.zeros((128, 128), dtype=np.float32)
    for m in range(128):
        half = (m // 64) * 64
        pi = half + ((m - half) + 32) % 64
        P[pi, m] = 1.0
    return P


def _prep_core(c, x, position_ids, attn_mask, Wqkv, bqkv, Wout, sinks, xT_full):
    s, h = c // 4, c % 4
    gstart = 1024 * s - 128
    xTc = np.zeros((E, TOK), dtype=np.float32)
    lo = max(0, gstart)
    xTc[:, lo - gstart:TOK] = xT_full[:, lo:gstart + TOK]
    qrows = np.arange(512 * h, 512 * h + 512)
    krows = np.arange(H * D + 128 * h, H * D + 128 * h + 128)
    vrows = np.arange((H + G) * D + 128 * h, (H + G) * D + 128 * h + 128)
    rows = np.concatenate([krows, vrows, qrows])   # feature order K, V, Q
    WqkvTc = np.ascontiguousarray(Wqkv[rows].T)
    bq = bqkv[rows].reshape(FTOT // 128, 128).T
    WoutTc = np.ascontiguousarray(Wout[:, 512 * h:512 * h + 512].T)
    cos2, sinS2 = _rope_tables(position_ids, gstart)
    masks = _build_masks01(s, gstart)
    # slot order within each 4-head group: [4g, 4g+2, 4g+1, 4g+3]
    slot_perm = [0, 2, 1, 3, 4, 6, 5, 7]
    esink = np.ascontiguousarray(
        np.broadcast_to(np.exp(sinks[0, 8 * h:8 * h + 8, 0, 0])[slot_perm][None, :],
                        (128, QH))).astype(np.float32)
    import ml_dtypes
    bf = ml_dtypes.bfloat16
    return {
        "xT": np.ascontiguousarray(xTc.astype(bf)),
        "wqkvT": np.ascontiguousarray(WqkvTc.astype(bf)),
        "bqkvT": np.ascontiguousarray(bq.astype(np.float32)),
        "woutT": np.ascontiguousarray(WoutTc.astype(bf)),
        "cosQ": np.ascontiguousarray((SCALE * cos2).astype(bf)),
        "sinQ": np.ascontiguousarray((SCALE * sinS2).astype(bf)),
        "cosK": np.ascontiguousarray(cos2.astype(bf)),
        "sinK": np.ascontiguousarray(sinS2.astype(bf)),
        "masks": masks,
        "esink": esink,
        "perm": _perm_matrix(),
    }


def _prep_all(inputs):
    x = np.asarray(inputs["x"], dtype=np.float32)
    position_ids = np.asarray(inputs["position_ids"])
    attn_mask = np.asarray(inputs["attn_mask"], dtype=np.float32)
    Wqkv = np.asarray(inputs["Wqkv"], dtype=np.float32)
    bqkv = np.asarray(inputs["bqkv"], dtype=np.float32)
    Wout = np.asarray(inputs["Wout"], dtype=np.float32)
    sinks = np.asarray(inputs["sinks"], dtype=np.float32)
    xT_full = np.ascontiguousarray(x[0].T)
    return [
        _prep_core(c, x, position_ids, attn_mask, Wqkv, bqkv, Wout, sinks, xT_full)
        for c in range(NCORES)
    ]


def kernel(x, position_ids, attn_mask, Wqkv, bqkv, Wout, bout, sinks):
    global _compiled
    from concourse.bass_utils import run_bass_kernel_spmd

    bout = np.asarray(bout, dtype=np.float32)

    if _compiled is None:
        _compiled = _build_bass()
    nc = _compiled

    in_maps = _prep_all({
        "x": x, "position_ids": position_ids, "attn_mask": attn_mask,
        "Wqkv": Wqkv, "bqkv": bqkv, "Wout": Wout, "bout": bout, "sinks": sinks,
    })
    res = run_bass_kernel_spmd(nc, in_maps, list(range(NCORES)))

    out = np.empty((S, E), dtype=np.float32)
    for s in range(2):
        acc = res.results[4 * s]["outT"].astype(np.float32)
        for h in range(1, 4):
            acc = acc + res.results[4 * s + h]["outT"].astype(np.float32)
        out[1024 * s:1024 * (s + 1)] = acc.T
    out += bout[None, :]
    return out[None]


# revision 40
# speedup vs baseline: 1.7823x; 1.2567x over previous
"""Trainium2 Bass kernel for nn_GroupQueryAttention_51616916963669.

GQA with YaRN RoPE, sliding-window (128) + causal mask, learned sink logit,
qkv/out projections. B=1, S=2048, E=2048, H=32, G=8, D=64.

Sharding over 8 NeuronCores: 2-way sequence (1024 queries each, with a
128-token KV halo) x 4-way heads (8 q-heads / 2 kv-groups each). Each core
computes a partial out-projection (over its 512 ctx dims); the host sums the
4 head-partials per sequence half and concatenates.

v4 design notes:
- Inputs arrive in ~12 large DMAs split across the two HWDGE queues (SP +
  ACT) -- per-chunk DMAs cost ~600ns dispatch each on the sync queue and
  serialized startup by ~20us in v3.
- Everything on the Q/K/V/probs path is bf16 (f32 only inside PSUM): DVE
  runs 2 elem/cycle/lane at 16 bit, and bf16 matmuls avoid the f32r
  partition-offset/PSUM-bank hazard.
- qkv matmul streams 3x384 columns per stationary load; RoPE rotate-half is
  a PE permutation matmul; the cos/sin multiplies run on DVE and the final
  adds (and KR partition-swap) on the otherwise idle GpSimd engine.
- Softmax: exp on ACT straight out of PSUM (2 ops per 4-head group); masked
  row-sums via fused DVE scalar_tensor_tensor with accum_out; normalization
  is one broadcast tensor_tensor; probs transposes on PE with the PSUM->SBUF
  copy on ACT; ctx parity copies on GpSimd.
- Out-projection is emitted after attention (engine queues overlap it with
  the attention tail), streams both query halves per stationary load, and
  writes bf16 via 16 output DMAs split across both queues.
"""
import os
import numpy as np

# ---- problem constants (hardcoded per contract) ----
B, S, E = 1, 2048, 2048
H, G, D = 32, 8, 64
SW = 128
ROPE_BASE = 10000.0
ORIG_CTX = 4096.0
YARN_SCALE = 2.0
BETA_FAST, BETA_SLOW = 32.0, 1.0

# ---- sharding constants ----
NCORES = 8
TOK = 1152           # local kv tokens (9 blocks of 128)
NQ = 1024            # local query tokens (kv blocks 1..8)
QH = 8               # q heads per core
KG = 2               # kv groups per core
FTOT = QH * D + 2 * KG * D   # 768, feature order [K, V, Q0..Q3]
NE = E // 128        # 16 e-chunks
TCH = 384            # qkv matmul N-chunk
NT = TOK // TCH      # 3
SCALE = 1.0 / (D ** 0.5)

_compiled = None


def _build_bass():
    import concourse.bacc as bacc
    import concourse.tile as tile
    import concourse.mybir as mybir
    from concourse.masks import make_identity

    f32 = mybir.dt.float32
    bf16 = mybir.dt.bfloat16
    Exp = mybir.ActivationFunctionType.Exp
    Ident = mybir.ActivationFunctionType.Identity
    Alu = mybir.AluOpType

    nc = bacc.Bacc("TRN2", target_bir_lowering=False, debug=False,
                   num_devices=NCORES)

    xT = nc.dram_tensor("xT", [E, TOK], bf16, kind="ExternalInput").ap()
    wqkvT = nc.dram_tensor("wqkvT", [E, FTOT], bf16, kind="ExternalInput").ap()
    bqkvT = nc.dram_tensor("bqkvT", [128, FTOT // 128], f32, kind="ExternalInput").ap()
    woutT = nc.dram_tensor("woutT", [QH * D, E], bf16, kind="ExternalInput").ap()
    cosQ = nc.dram_tensor("cosQ", [128, TOK], bf16, kind="ExternalInput").ap()
    sinQ = nc.dram_tensor("sinQ", [128, TOK], bf16, kind="ExternalInput").ap()
    cosK = nc.dram_tensor("cosK", [128, TOK], bf16, kind="ExternalInput").ap()
    sinK = nc.dram_tensor("sinK", [128, TOK], bf16, kind="ExternalInput").ap()
    masksD = nc.dram_tensor("masks", [128, 2, 256], bf16, kind="ExternalInput").ap()
    esinkD = nc.dram_tensor("esink", [128, QH], f32, kind="ExternalInput").ap()
    permD = nc.dram_tensor("perm", [128, 128], bf16, kind="ExternalInput").ap()
    outT = nc.dram_tensor("outT", [E, NQ], bf16, kind="ExternalOutput").ap()

    xT_r = xT.rearrange("(a p) t -> p a t", p=128)        # [128, 16, TOK]
    wq_r = wqkvT.rearrange("(a p) f -> p a f", p=128)     # [128, 16, 768]
    wo_r = woutT.rearrange("(a p) e -> p a e", p=128)     # [128, 4, 2048]
    out_r = outT.rearrange("(a p) q -> p a q", p=128)     # [128, 16, 1024]

    with tile.TileContext(nc) as tc:
        from contextlib import ExitStack
        es = ExitStack()
        with es:
            persist = es.enter_context(tc.tile_pool(name="persist", bufs=1))
            qk_pool = es.enter_context(tc.tile_pool(name="qk", bufs=1))
            ctx_pool = es.enter_context(tc.tile_pool(name="ctx", bufs=1))
            qkv_pool = es.enter_context(tc.tile_pool(name="qkv", bufs=1))
            inp_pool = es.enter_context(tc.tile_pool(name="inp", bufs=1))

            # ---- inputs: few big DMAs, split across both HWDGE queues ----
            W_sb = inp_pool.tile([128, NE, FTOT], bf16)
            x_sb = inp_pool.tile([128, NE, TOK], bf16)
            for fg in range(3):
                nc.sync.dma_start(W_sb[:, :, 256 * fg:256 * (fg + 1)],
                                  wq_r[:, :, 256 * fg:256 * (fg + 1)])
            cs_t = {}
            for eq in range(4):
                nc.scalar.dma_start(x_sb[:, 4 * eq:4 * eq + 4, :],
                                    xT_r[:, 4 * eq:4 * eq + 4, :])
                if eq == 0:
                    for nm, src in (("cK", cosK), ("sK", sinK)):
                        t = persist.tile([128, TOK], bf16, tag=nm, name=nm)
                        nc.scalar.dma_start(t, src)
                        cs_t[nm] = t
            for nm, src in (("cQ", cosQ), ("sQ", sinQ)):
                t = persist.tile([128, TOK], bf16, tag=nm, name=nm)
                nc.scalar.dma_start(t, src)
                cs_t[nm] = t

            ident = persist.tile([128, 128], f32)
            make_identity(nc, ident)
            identb = persist.tile([128, 128], bf16)
            nc.vector.tensor_copy(identb, ident)
            b_sb = persist.tile([128, FTOT // 128], f32)
            nc.sync.dma_start(b_sb, bqkvT)
            masks2 = persist.tile([128, 2, 256], bf16)
            nc.sync.dma_start(masks2, masksD)
            es_sink = persist.tile([128, QH], f32)
            nc.sync.dma_start(es_sink, esinkD)
            perm = persist.tile([128, 128], bf16)
            nc.sync.dma_start(perm, permD)
            Wo = persist.tile([128, 4, E], bf16)
            nc.sync.dma_start(Wo, wo_r)

            # qkv projection results (feature blocks: 0=K, 1=V, 2..5=Q0..Q3)
            qkvT_t = [qkv_pool.tile([128, TOK], bf16, tag=f"qkvT{i}",
                                    name=f"qkvT{i}") for i in range(6)]
            QR = [qk_pool.tile([128, TOK], bf16, tag=f"QR{i}", name=f"QR{i}")
                  for i in range(4)]
            KR = qk_pool.tile([128, TOK], bf16, tag="KR")
            KRsw = qk_pool.tile([128, TOK], bf16, tag="KRsw")
            Vtok = ctx_pool.tile([128, 9, KG, D], bf16)
            # ctx transposed: [128 part = pair of heads, pair-idx 4, q-half 2, 512]
            ctxT = ctx_pool.tile([128, 4, 2, 512], bf16)

            esA = ExitStack()
            psA = esA.enter_context(
                tc.tile_pool(name="psA", bufs=2, space="PSUM"))
            psR = esA.enter_context(
                tc.tile_pool(name="psR", bufs=2, space="PSUM"))
            rope_sc = es.enter_context(tc.tile_pool(name="ropesc", bufs=3))

            def qkv_block(f):
                """Accumulate feature block f over all 16 e-chunks; one
                stationary load per (e), streaming 3x384 columns."""
                pst = [psA.tile([128, TCH], f32, tag=f"mmA{t}", name=f"mmA{t}")
                       for t in range(NT)]
                for e in range(NE):
                    for t in range(NT):
                        nc.tensor.matmul(
                            pst[t], W_sb[:, e, 128 * f:128 * (f + 1)],
                            x_sb[:, e, TCH * t:TCH * (t + 1)],
                            start=(e == 0), stop=(e == NE - 1))
                for t in range(NT):
                    nc.scalar.activation(
                        out=qkvT_t[f][:, TCH * t:TCH * (t + 1)], in_=pst[t],
                        func=Ident, bias=b_sb[:, f:f + 1])

            def rope(f, cT, sT, dst, also_swap=None):
                """dst = qkvT[f]*cos + (perm @ qkvT[f])*sinS, in 384-col
                chunks. Rotate-half on PE; muls on DVE; adds on GpSimd."""
                src = qkvT_t[f]
                for t in range(NT):
                    cs_ = slice(TCH * t, TCH * (t + 1))
                    rot = psR.tile([128, TCH], f32, tag="rot", name="rot")
                    nc.tensor.matmul(rot, perm, src[:, cs_],
                                     start=True, stop=True)
                    m1 = rope_sc.tile([128, TCH], bf16, tag="m1", name="m1")
                    nc.vector.tensor_mul(m1, src[:, cs_], cT[:, cs_])
                    m2 = rope_sc.tile([128, TCH], bf16, tag="m2", name="m2")
                    nc.vector.tensor_mul(m2, rot, sT[:, cs_])
                    nc.gpsimd.tensor_add(dst[:, cs_], m1, m2)
                    if also_swap is not None:
                        nc.gpsimd.tensor_add(
                            also_swap[0:64, cs_], m1[64:128, :], m2[64:128, :])
                        nc.gpsimd.tensor_add(
                            also_swap[64:128, cs_], m1[0:64, :], m2[0:64, :])

            def v_transpose():
                V = qkvT_t[1]
                for k in range(9):
                    for g in range(KG):
                        pt = psR.tile([128, TCH], f32, tag="rot", name="vt")
                        ptb = pt.bitcast(bf16)
                        nc.tensor.transpose(
                            ptb[:, 0:D],
                            V[64 * g:64 * (g + 1), 128 * k:128 * (k + 1)],
                            identb[64 * g:64 * (g + 1), 64 * g:64 * (g + 1)])
                        nc.vector.tensor_copy(Vtok[:, k, g, :], ptb[:, 0:D])

            pb = es.enter_context(tc.tile_pool(name="phB", bufs=2))
            pbt = es.enter_context(tc.tile_pool(name="phBt", bufs=4))
            psB = psBc = psBt = None

            def attn_group(qb, g):
                """One 4-head group (kv group g) for query block qb.
                Slot order [4g, 4g+2, 4g+1, 4g+3]: each PSUM bank gets a
                same-half pair of scores matmuls."""
                sc4 = psB.tile([128, 4, 256], f32, tag="sc4", name="sc4")
                for slot in range(4):
                    half = slot // 2
                    pair = 2 * g + (slot % 2)
                    ktile = KR if (g == half) else KRsw
                    qsl = QR[pair][64 * half:64 * (half + 1), :]
                    ksl = ktile[64 * half:64 * (half + 1), :]
                    nc.tensor.matmul(
                        sc4[:, slot, :],
                        qsl[:, 128 * (qb + 1):128 * (qb + 2)],
                        ksl[:, 128 * qb:128 * qb + 256],
                        start=True, stop=True)
                pS4 = pb.tile([128, 4, 256], bf16, tag="pS4", name="pS4")
                nc.scalar.activation(out=pS4[:, 0:2, :], in_=sc4[:, 0:2, :],
                                     func=Exp)
                nc.scalar.activation(out=pS4[:, 2:4, :], in_=sc4[:, 2:4, :],
                                     func=Exp)
                pM4 = pb.tile([128, 4, 256], bf16, tag="pM4", name="pM4")
                rs4 = pb.tile([128, 4], f32, tag="rs4", name="rs4")
                mk = masks2[:, min(qb, 1), :]
                for j in range(4):
                    nc.vector.scalar_tensor_tensor(
                        out=pM4[:, j, :], in0=pS4[:, j, :], scalar=0.0,
                        in1=mk, op0=Alu.bypass, op1=Alu.mult,
                        accum_out=rs4[:, j:j + 1])
                dn4 = pb.tile([128, 4], f32, tag="dn4", name="dn4")
                nc.vector.tensor_add(dn4, rs4, es_sink[:, 4 * g:4 * g + 4])
                rinv4 = pb.tile([128, 4], f32, tag="rinv4", name="rinv4")
                nc.vector.reciprocal(rinv4, dn4)
                pN4 = pb.tile([128, 4, 256], bf16, tag="pN4", name="pN4")
                nc.vector.tensor_tensor(
                    out=pN4, in0=pM4,
                    in1=rinv4.unsqueeze(2).broadcast_to([128, 4, 256]),
                    op=Alu.mult)
                pT4 = pbt.tile([128, 4, 2, 128], bf16, tag="pT4", name="pT4")
                tp4 = psBt.tile([128, 4, 2, 128], bf16, tag="tp4", name="tp4")
                for j in range(4):
                    for bk in range(2):
                        nc.tensor.transpose(
                            tp4[:, j, bk, :],
                            pN4[:, j, 128 * bk:128 * (bk + 1)], identb)
                nc.vector.tensor_copy(pT4, tp4)
                cps4 = psBc.tile([64, 4, 128], f32, tag="cps4", name="cps4")
                for j in range(4):
                    for bk in range(2):
                        nc.tensor.matmul(
                            cps4[:, j, :], Vtok[:, qb + bk, g, :],
                            pT4[:, j, bk, :],
                            start=(bk == 0), stop=(bk == 1))
                th, qq = qb // 4, qb % 4
                nc.scalar.activation(
                    out=ctxT[0:64, 2 * g:2 * g + 2, th, 128 * qq:128 * (qq + 1)],
                    in_=cps4[:, 0:2, :], func=Ident)
                nc.scalar.activation(
                    out=ctxT[64:128, 2 * g:2 * g + 2, th, 128 * qq:128 * (qq + 1)],
                    in_=cps4[:, 2:4, :], func=Ident)

            pco = es.enter_context(tc.tile_pool(name="phCo", bufs=3))
            psC = None

            def outproj():
                for e in range(NE):
                    pst = [psC.tile([128, 512], f32, tag=f"mmC{t}",
                                    name=f"mmC{t}") for t in range(2)]
                    for h4 in range(4):
                        for t in range(2):
                            nc.tensor.matmul(
                                pst[t], Wo[:, h4, 128 * e:128 * (e + 1)],
                                ctxT[:, h4, t, :],
                                start=(h4 == 0), stop=(h4 == 3))
                    o_sb = pco.tile([128, 2, 512], bf16, tag="o", name="o")
                    for t in range(2):
                        nc.scalar.activation(out=o_sb[:, t, :], in_=pst[t],
                                             func=Ident)
                    eng = nc.sync if e % 2 == 0 else nc.scalar
                    eng.dma_start(out_r[:, e, :], o_sb)

            # ---------- emission schedule ----------
            qkv_block(0)                                   # K
            rope(0, cs_t["cK"], cs_t["sK"], KR, also_swap=KRsw)
            qkv_block(1)                                   # V
            v_transpose()
            qkv_block(2)                                   # Q0
            rope(2, cs_t["cQ"], cs_t["sQ"], QR[0])
            qkv_block(3)                                   # Q1
            rope(3, cs_t["cQ"], cs_t["sQ"], QR[1])
            qkv_block(4)                                   # Q2
            rope(4, cs_t["cQ"], cs_t["sQ"], QR[2])
            qkv_block(5)                                   # Q3
            rope(5, cs_t["cQ"], cs_t["sQ"], QR[3])
            esA.close()                                    # free A PSUM banks

            esB = ExitStack()
            psB = esB.enter_context(
                tc.tile_pool(name="psB", bufs=2, space="PSUM"))
            psBc = esB.enter_context(
                tc.tile_pool(name="psBc", bufs=2, space="PSUM"))
            psBt = esB.enter_context(
                tc.tile_pool(name="psBt", bufs=2, space="PSUM"))

            for qb in range(8):
                attn_group(qb, 0)
            for qb in range(8):
                attn_group(qb, 1)
            esB.close()                                    # free B PSUM banks

            psC = es.enter_context(
                tc.tile_pool(name="psC", bufs=2, space="PSUM"))
            outproj()

    nc.compile()
    return nc


# ---------------- host-side prep ----------------

def _rope_tables(position_ids, gstart):
    pos = np.zeros(TOK, dtype=np.float32)
    idx = gstart + np.arange(TOK)
    valid = (idx >= 0) & (idx < S)
    pos[valid] = position_ids[0, idx[valid]].astype(np.float32)
    freqs = (1.0 / ROPE_BASE ** (np.arange(0, D, 2, dtype=np.float32) / D)).astype(np.float32)
    wave_len = 2.0 * np.pi / freqs
    low = ORIG_CTX / BETA_FAST
    high = ORIG_CTX / BETA_SLOW
    t = np.clip((wave_len - low) / (high - low), 0.0, 1.0)
    eff = freqs * (1.0 - t) + (freqs / YARN_SCALE) * t
    conc = 0.1 * np.log(np.float32(YARN_SCALE)) + 1.0
    ang = pos[:, None] * eff[None, :] * conc
    sin = np.sin(ang).astype(np.float32).T    # [32, TOK]
    cos = np.cos(ang).astype(np.float32).T
    cosT = np.concatenate([cos, cos], axis=0)  # [64, TOK]
    sinS = np.concatenate([-sin, sin], axis=0)
    cos2 = np.concatenate([cosT, cosT], axis=0)  # [128, TOK]
    sinS2 = np.concatenate([sinS, sinS], axis=0)
    return np.ascontiguousarray(cos2), np.ascontiguousarray(sinS2)


def _build_masks01(s, gstart):
    """Multiplicative 0/1 band mask, [128, 2, 256] (qb==0 variant, qb>=1)."""
    qb = np.arange(2)[None, :, None]
    il = np.arange(128)[:, None, None]
    j = np.arange(256)[None, None, :]
    gq = 1024 * s + 128 * qb + il
    gk = gstart + 128 * qb + j
    gq_b, gk_b = np.broadcast_arrays(gq, gk)
    valid = (gk_b >= 0) & (gk_b <= gq_b) & (gk_b > gq_b - SW)
    return np.ascontiguousarray(valid.astype(np.float32))


def _perm_matrix():
    """lhsT for rotate-half: out[p] = src[p xor 32] within each 64-half."""
    P = np.zeros((128, 128), dtype=np.float32)
    for m in range(128):
        half = (m // 64) * 64
        pi = half + ((m - half) + 32) % 64
        P[pi, m] = 1.0
    return P


def _prep_core(c, x, position_ids, attn_mask, Wqkv, bqkv, Wout, sinks, xT_full):
    s, h = c // 4, c % 4
    gstart = 1024 * s - 128
    xTc = np.zeros((E, TOK), dtype=np.float32)
    lo = max(0, gstart)
    xTc[:, lo - gstart:TOK] = xT_full[:, lo:gstart + TOK]
    qrows = np.arange(512 * h, 512 * h + 512)
    krows = np.arange(H * D + 128 * h, H * D + 128 * h + 128)
    vrows = np.arange((H + G) * D + 128 * h, (H + G) * D + 128 * h + 128)
    rows = np.concatenate([krows, vrows, qrows])   # feature order K, V, Q
    WqkvTc = np.ascontiguousarray(Wqkv[rows].T)
    bq = bqkv[rows].reshape(FTOT // 128, 128).T
    WoutTc = np.ascontiguousarray(Wout[:, 512 * h:512 * h + 512].T)
    cos2, sinS2 = _rope_tables(position_ids, gstart)
    masks = _build_masks01(s, gstart)
    # slot order within each 4-head group: [4g, 4g+2, 4g+1, 4g+3]
    slot_perm = [0, 2, 1, 3, 4, 6, 5, 7]
    esink = np.ascontiguousarray(
        np.broadcast_to(np.exp(sinks[0, 8 * h:8 * h + 8, 0, 0])[slot_perm][None, :],
                        (128, QH))).astype(np.float32)
    import ml_dtypes
    bf = ml_dtypes.bfloat16
    return {
        "xT": np.ascontiguousarray(xTc.astype(bf)),
        "wqkvT": np.ascontiguousarray(WqkvTc.astype(bf)),
        "bqkvT": np.ascontiguousarray(bq.astype(np.float32)),
        "woutT": np.ascontiguousarray(WoutTc.astype(bf)),
        "cosQ": np.ascontiguousarray((SCALE * cos2).astype(bf)),
        "sinQ": np.ascontiguousarray((SCALE * sinS2).astype(bf)),
        "cosK": np.ascontiguousarray(cos2.astype(bf)),
        "sinK": np.ascontiguousarray(sinS2.astype(bf)),
        "masks": np.ascontiguousarray(masks.astype(bf)),
        "esink": esink,
        "perm": np.ascontiguousarray(_perm_matrix().astype(bf)),
    }


def _prep_all(inputs):
    x = np.asarray(inputs["x"], dtype=np.float32)
    position_ids = np.asarray(inputs["position_ids"])
    attn_mask = np.asarray(inputs["attn_mask"], dtype=np.float32)
    Wqkv = np.asarray(inputs["Wqkv"], dtype=np.float32)
    bqkv = np.asarray(inputs["bqkv"], dtype=np.float32)
    Wout = np.asarray(inputs["Wout"], dtype=np.float32)
    sinks = np.asarray(inputs["sinks"], dtype=np.float32)
    xT_full = np.ascontiguousarray(x[0].T)
    return [
        _prep_core(c, x, position_ids, attn_mask, Wqkv, bqkv, Wout, sinks, xT_full)
        for c in range(NCORES)
    ]


def kernel(x, position_ids, attn_mask, Wqkv, bqkv, Wout, bout, sinks):
    global _compiled
    from concourse.bass_utils import run_bass_kernel_spmd

    bout = np.asarray(bout, dtype=np.float32)

    if _compiled is None:
        _compiled = _build_bass()
    nc = _compiled

    in_maps = _prep_all({
        "x": x, "position_ids": position_ids, "attn_mask": attn_mask,
        "Wqkv": Wqkv, "bqkv": bqkv, "Wout": Wout, "bout": bout, "sinks": sinks,
    })
    res = run_bass_kernel_spmd(nc, in_maps, list(range(NCORES)))

    out = np.empty((S, E), dtype=np.float32)
    for s in range(2):
        acc = res.results[4 * s]["outT"].astype(np.float32)
        for h in range(1, 4):
            acc = acc + res.results[4 * s + h]["outT"].astype(np.float32)
        out[1024 * s:1024 * (s + 1)] = acc.T
    out += bout[None, :]
    return out[None]
